# revision 10
# baseline (speedup 1.0000x reference)
"""AttnBlock (GroupNorm -> q/k/v 1x1 conv -> single-head attention -> proj -> residual)
on 8 Trainium2 NeuronCores.

Sharding: pure data-parallel over batch. x is [B=8, C=512, N=2048]; core b runs the
full attention block on x[b]. No collectives.

Per-core dataflow (all matmuls in float32r -> full PE rate at free-dim 512):
  - GroupNorm(32 groups): per-row bn_stats/bn_aggr, cross-partition group reduce and
    broadcast via tiny PE matmuls with 0/1 selector matrices.
  - q/k projections into [c, n] layout; v projection directly into transposed [m, c]
    layout (vT = h.T @ wv.T) so the PV matmul needs no extra transpose of v.
  - S = q.T k per 128-query tile, softmax without max-subtraction (logits ~ N(0,1),
    fp32-safe), exp on ScalarE with accum_out row-sums.
  - P transposed into [m, n] tiles via PE transpose; 1/rowsum folded into the PV
    PSUM->SBUF copy through a PE-broadcast reciprocal tile.
  - Output projection + bias + residual fused per 512-query block.
"""

import sys

sys.path.insert(0, "/opt/trn_rl_repo")

from contextlib import ExitStack

import numpy as np

import concourse.bass as bass
import concourse.bacc as bacc
import concourse.tile as tile
from concourse import mybir
from concourse.bass_utils import run_bass_kernel_spmd

P = 128
C = 512
N = 2048
B = 8
GROUPS = 32
GSZ = 16  # channels (partition rows) per group
GPT = P // GSZ  # groups per 128-channel tile = 8
CT = C // P  # 4 channel tiles
NBLK = N // 512  # 4 query blocks of 512
MT = N // P  # 16 key tiles of 128
EPS = 1e-6
SCALE = float(C) ** -0.5

f32 = mybir.dt.float32
f32r = mybir.dt.float32r
AX = mybir.AxisListType
OP = mybir.AluOpType
AF = mybir.ActivationFunctionType


def _r(ap):
    """Bitcast an fp32 AP to float32r for the tensor engine."""
    return ap.bitcast(f32r)


def build():
    nc = bacc.Bacc()

    x_d = nc.declare_dram_parameter("x", [C, N], f32, False)
    gns_d = nc.declare_dram_parameter("gn_scale", [C], f32, False)
    gnb_d = nc.declare_dram_parameter("gn_bias", [C], f32, False)
    w_d = {}
    b_d = {}
    for nm in ("wq", "wk", "wv", "wp"):
        w_d[nm] = nc.declare_dram_parameter(nm, [C, C], f32, False)
    for nm in ("bq", "bk", "bv", "bp"):
        b_d[nm] = nc.declare_dram_parameter(nm, [C], f32, False)
    sel_d = nc.declare_dram_parameter("sel", [P, GPT], f32, False)
    selT_d = nc.declare_dram_parameter("selT", [GPT, P], f32, False)
    id_d = nc.declare_dram_parameter("ident", [P, P], f32, False)
    out_d = nc.declare_dram_parameter("out", [C, N], f32, True)

    with ExitStack() as ctx:
        tc = ctx.enter_context(tile.TileContext(nc))

        const = ctx.enter_context(tc.tile_pool(name="const", bufs=1))

        ident = const.tile([P, P], f32, tag="ident")
        nc.sync.dma_start(out=ident, in_=id_d[:, :])
        ident_r = const.tile([P, P], f32r, tag="ident_r")
        nc.vector.tensor_copy(out=ident_r, in_=ident)
        ident_c = const.tile([P, P], f32, tag="ident_c")
        nc.vector.tensor_copy(out=ident_c, in_=ident)
        sel_sb = const.tile([P, GPT], f32, tag="sel")
        nc.sync.dma_start(out=sel_sb, in_=sel_d[:, :])
        sel_r = const.tile([P, GPT], f32r, tag="sel_r")
        nc.vector.tensor_copy(out=sel_r, in_=sel_sb)
        selT_sb = const.tile([GPT, P], f32, tag="selT")
        nc.sync.dma_start(out=selT_sb, in_=selT_d[:, :])
        selT_r = const.tile([GPT, P], f32r, tag="selT_r")
        nc.vector.tensor_copy(out=selT_r, in_=selT_sb)
        ones_sb = const.tile([1, P], f32, tag="ones")
        nc.vector.memset(ones_sb, 1.0)
        eps_sb = const.tile([GPT, 1], f32, tag="eps")
        nc.vector.memset(eps_sb, EPS)

        gs_sb = const.tile([P, CT], f32, tag="gs")
        gb_sb = const.tile([P, CT], f32, tag="gb")
        bq_sb = const.tile([P, CT], f32, tag="bq")
        bk_sb = const.tile([P, CT], f32, tag="bk")
        bp_sb = const.tile([P, CT], f32, tag="bp")
        for ci in range(CT):
            sl = slice(ci * P, (ci + 1) * P)
            nc.sync.dma_start(out=gs_sb[:, ci : ci + 1], in_=gns_d[sl].unsqueeze(1))
            nc.sync.dma_start(out=gb_sb[:, ci : ci + 1], in_=gnb_d[sl].unsqueeze(1))
            nc.sync.dma_start(out=bq_sb[:, ci : ci + 1], in_=b_d["bq"][sl].unsqueeze(1))
            nc.sync.dma_start(out=bk_sb[:, ci : ci + 1], in_=b_d["bk"][sl].unsqueeze(1))
            nc.sync.dma_start(out=bp_sb[:, ci : ci + 1], in_=b_d["bp"][sl].unsqueeze(1))
        bv_sb = const.tile([P, C], f32, tag="bv")
        nc.sync.dma_start(out=bv_sb, in_=b_d["bv"][:].unsqueeze(0).to_broadcast([P, C]))

        # ---- weights: load [o, c] tiles and PE-transpose into [c, o] tiles ----
        wT = {}
        wt_p_pool = ctx.enter_context(tc.tile_pool(name="wt_p", bufs=1))
        qk_pool = ctx.enter_context(tc.tile_pool(name="qk", bufs=1))
        vt_pool = ctx.enter_context(tc.tile_pool(name="vt", bufs=1))
        psum_mm = ctx.enter_context(tc.tile_pool(name="psum_mm", bufs=4, space="PSUM"))
        qkv_scope = ExitStack()
        wt_qkv_pool = qkv_scope.enter_context(tc.tile_pool(name="wt_qkv", bufs=1))
        with tc.tile_pool(name="wraw", bufs=4) as wraw, tc.tile_pool(
            name="psum_w", bufs=2, space="PSUM"
        ) as psum_w:
            for nm in ("wq", "wk", "wv", "wp"):
                pool_w = wt_p_pool if nm == "wp" else wt_qkv_pool
                wT[nm] = [
                    pool_w.tile([P, C], f32r, tag=f"wT_{nm}{ci}", name=f"wT_{nm}{ci}") for ci in range(CT)
                ]
                for oi in range(CT):
                    raw = wraw.tile([P, C], f32, tag="wraw", bufs=16)
                    nc.sync.dma_start(out=raw, in_=w_d[nm][oi * P : (oi + 1) * P, :])
                    raw_r = wraw.tile([P, C], f32r, tag="wraw_r")
                    nc.vector.tensor_copy(out=raw_r, in_=raw)
                    for ci in range(CT):
                        ps = psum_w.tile([P, P], f32r, tag="ps_w")
                        nc.tensor.transpose(
                            ps, raw_r[:, ci * P : (ci + 1) * P], ident_r
                        )
                        dst = wT[nm][ci][:, oi * P : (oi + 1) * P]
                        if ci % 2 == 0:
                            nc.vector.tensor_copy(out=dst, in_=ps)
                        else:
                            nc.scalar.copy(out=dst, in_=ps)

        # ---- GroupNorm ----
        h_pool = qkv_scope.enter_context(tc.tile_pool(name="h", bufs=1))
        h = []
        with tc.tile_pool(name="xg", bufs=1) as xg_pool, tc.tile_pool(
            name="gn_tmp", bufs=4
        ) as gn_tmp, tc.tile_pool(name="psum_gn", bufs=2, space="PSUM") as psum_gn:
            stats4 = const.tile([P, 2 * CT], f32, tag="stats4")
            xg = []
            for ci in range(CT):
                xt = xg_pool.tile([P, N], f32, tag=f"xg{ci}")
                nc.sync.dma_start(out=xt, in_=x_d[ci * P : (ci + 1) * P, :])
                xg.append(xt)
                st = gn_tmp.tile([P, 4, 6], f32, tag="st")
                for j in range(4):
                    nc.vector.bn_stats(out=st[:, j, :], in_=xt[:, j * 512 : (j + 1) * 512])
                mv = gn_tmp.tile([P, 2], f32, tag="mv")
                nc.vector.bn_aggr(out=mv, in_=st)
                nc.vector.tensor_copy(out=stats4[:, ci : ci + 1], in_=mv[:, 0:1])
                # E[x^2] = mean^2 + var
                nc.vector.tensor_tensor(
                    out=stats4[:, CT + ci : CT + ci + 1],
                    in0=mv[:, 0:1],
                    in1=mv[:, 0:1],
                    op=OP.mult,
                )
                nc.vector.tensor_add(
                    out=stats4[:, CT + ci : CT + ci + 1],
                    in0=stats4[:, CT + ci : CT + ci + 1],
                    in1=mv[:, 1:2],
                )
            # group-aggregate across the 16-row groups of each tile
            stats4_r = const.tile([P, 2 * CT], f32r, tag="stats4_r")
            nc.vector.tensor_copy(out=stats4_r, in_=stats4)
            psg = psum_gn.tile([GPT, 2 * CT], f32, tag="psg")
            nc.tensor.matmul(psg, sel_r, stats4_r, start=True, stop=True)
            g2 = const.tile([GPT, 2 * CT], f32, tag="g2")
            gtmp = const.tile([GPT, 2 * CT], f32, tag="gtmp")
            nc.vector.tensor_scalar_mul(g2[:, 0:CT], psg[:, 0:CT], 1.0 / GSZ)
            nc.vector.tensor_scalar_mul(gtmp[:, 0:CT], psg[:, CT : 2 * CT], 1.0 / GSZ)
            nc.vector.tensor_tensor(
                out=gtmp[:, CT : 2 * CT], in0=g2[:, 0:CT], in1=g2[:, 0:CT], op=OP.mult
            )
            nc.vector.tensor_sub(gtmp[:, 0:CT], gtmp[:, 0:CT], gtmp[:, CT : 2 * CT])
            nc.scalar.activation(
                out=gtmp[:, 0:CT], in_=gtmp[:, 0:CT], func=AF.Sqrt, bias=eps_sb, scale=1.0
            )
            nc.vector.reciprocal(out=g2[:, CT : 2 * CT], in_=gtmp[:, 0:CT])
            # broadcast per-group stats back to the 128 rows of each tile
            g2_r = const.tile([GPT, 2 * CT], f32r, tag="g2_r")
            nc.vector.tensor_copy(out=g2_r, in_=g2)
            psb = psum_gn.tile([P, 2 * CT], f32, tag="psb")
            nc.tensor.matmul(psb, selT_r, g2_r, start=True, stop=True)
            rowst = const.tile([P, 2 * CT], f32, tag="rowst")
            nc.vector.tensor_copy(out=rowst, in_=psb)
            # fold gn scale/bias: h = A*x + B with A = rstd*scale, B = bias - mean*A
            AB = const.tile([P, 2 * CT], f32, tag="AB")
            for ci in range(CT):
                nc.vector.tensor_tensor(
                    out=AB[:, ci : ci + 1],
                    in0=rowst[:, CT + ci : CT + ci + 1],
                    in1=gs_sb[:, ci : ci + 1],
                    op=OP.mult,
                )
                nc.vector.tensor_tensor(
                    out=AB[:, CT + ci : CT + ci + 1],
                    in0=rowst[:, ci : ci + 1],
                    in1=AB[:, ci : ci + 1],
                    op=OP.mult,
                )
                nc.vector.tensor_sub(
                    AB[:, CT + ci : CT + ci + 1],
                    gb_sb[:, ci : ci + 1],
                    AB[:, CT + ci : CT + ci + 1],
                )
            for ci in range(CT):
                ht = h_pool.tile([P, N], f32r, tag=f"h{ci}")
                nc.vector.tensor_scalar(
                    out=ht,
                    in0=xg[ci],
                    scalar1=AB[:, ci : ci + 1],
                    scalar2=AB[:, CT + ci : CT + ci + 1],
                    op0=OP.mult,
                    op1=OP.add,
                )
                h.append(ht)

        # ---- projections ----
        q = [qk_pool.tile([P, N], f32r, tag=f"q{ci}", name=f"q{ci}") for ci in range(CT)]
        k = [qk_pool.tile([P, N], f32r, tag=f"k{ci}", name=f"k{ci}") for ci in range(CT)]
        for nm, dst, bias_sb in (("wq", q, bq_sb), ("wk", k, bk_sb)):
            for oi in range(CT):
                for nb in range(NBLK):
                    ps = psum_mm.tile([P, 512], f32, tag="ps_mm")
                    for ci in range(CT):
                        nc.tensor.matmul(
                            ps,
                            wT[nm][ci][:, oi * P : (oi + 1) * P],
                            h[ci][:, nb * 512 : (nb + 1) * 512],
                            start=(ci == 0),
                            stop=(ci == CT - 1),
                        )
                    nc.vector.tensor_scalar_add(
                        dst[oi][:, nb * 512 : (nb + 1) * 512],
                        ps,
                        bias_sb[:, oi : oi + 1],
                    )
        vT = []
        for mi in range(MT):
            ps = psum_mm.tile([P, 512], f32, tag="ps_mm")
            for ci in range(CT):
                nc.tensor.matmul(
                    ps,
                    h[ci][:, mi * P : (mi + 1) * P],
                    wT["wv"][ci],
                    start=(ci == 0),
                    stop=(ci == CT - 1),
                )
            vt = vt_pool.tile([P, C], f32r, tag=f"vT{mi}")
            nc.vector.tensor_add(out=vt, in0=ps, in1=bv_sb)
            vT.append(vt)
        qkv_scope.close()

        # ---- attention + fused output projection ----
        with tc.tile_pool(name="p_big", bufs=2) as p_pool, tc.tile_pool(
            name="ptr", bufs=16
        ) as pt_pool, tc.tile_pool(name="sm", bufs=2) as sm_pool, tc.tile_pool(
            name="h2", bufs=4
        ) as h2_pool, tc.tile_pool(name="xres", bufs=4) as xres_pool, tc.tile_pool(
            name="outp", bufs=2
        ) as out_pool, tc.tile_pool(
            name="psum_tr", bufs=2, space="PSUM"
        ) as psum_tr, tc.tile_pool(name="psum_pv", bufs=2, space="PSUM") as psum_pv:
            for qb in range(NBLK):
                x_res = []
                for oi in range(CT):
                    xr = xres_pool.tile([P, 512], f32, tag="xres")
                    nc.gpsimd.dma_start(
                        out=xr,
                        in_=x_d[oi * P : (oi + 1) * P, qb * 512 : (qb + 1) * 512],
                    )
                    x_res.append(xr)
                PT = [pt_pool.tile([P, 512], f32r, tag="pt", name="pt") for _ in range(MT)]
                rsumT = sm_pool.tile([1, 512], f32, tag="rsumT")
                for nt in range(4):
                    n0 = qb * 512 + nt * P
                    ps_s = [psum_mm.tile([P, 512], f32, tag="ps_mm", name="ps_s") for _ in range(4)]
                    for mb in range(4):
                        for ci in range(CT):
                            nc.tensor.matmul(
                                ps_s[mb],
                                q[ci][:, n0 : n0 + P],
                                k[ci][:, mb * 512 : (mb + 1) * 512],
                                start=(ci == 0),
                                stop=(ci == CT - 1),
                            )
                    Pt = p_pool.tile([P, N], f32r, tag="p_big")
                    ssum_t = sm_pool.tile([P, 4], f32, tag="ssum", bufs=4)
                    for mb in range(4):
                        nc.scalar.activation(
                            out=Pt[:, mb * 512 : (mb + 1) * 512],
                            in_=ps_s[mb],
                            func=AF.Exp,
                            scale=SCALE,
                            accum_out=ssum_t[:, mb : mb + 1],
                        )
                    ssum1 = sm_pool.tile([P, 1], f32, tag="ssum1")
                    nc.vector.reduce_sum(out=ssum1, in_=ssum_t, axis=AX.X)
                    ps_r1 = psum_tr.tile([1, P], f32, tag="ps_tr")
                    nc.tensor.transpose(ps_r1, ssum1, ident_c)
                    nc.vector.tensor_copy(
                        out=rsumT[:, nt * P : (nt + 1) * P], in_=ps_r1
                    )
                    for mi in range(MT):
                        ps_tr = psum_tr.tile([P, P], f32r, tag="ps_tr")
                        nc.tensor.transpose(
                            ps_tr, Pt[:, mi * P : (mi + 1) * P], ident_r
                        )
                        dst = PT[mi][:, nt * P : (nt + 1) * P]
                        if mi % 2 == 0:
                            nc.vector.tensor_copy(out=dst, in_=ps_tr)
                        else:
                            nc.scalar.copy(out=dst, in_=ps_tr)
                # 1/rowsum, broadcast across partitions via PE
                rinv = sm_pool.tile([1, 512], f32, tag="rinv")
                nc.vector.reciprocal(out=rinv, in_=rsumT)
                ps_R = psum_mm.tile([P, 512], f32, tag="ps_mm")
                nc.tensor.matmul(ps_R, ones_sb, rinv, start=True, stop=True)
                Rsb = sm_pool.tile([P, 512], f32, tag="Rsb")
                nc.vector.tensor_copy(out=Rsb, in_=ps_R)
                # PV
                h2 = []
                for ci in range(CT):
                    ps_pv = psum_pv.tile([P, 512], f32, tag="ps_pv")
                    for mi in range(MT):
                        nc.tensor.matmul(
                            ps_pv,
                            vT[mi][:, ci * P : (ci + 1) * P],
                            PT[mi],
                            start=(mi == 0),
                            stop=(mi == MT - 1),
                        )
                    h2t = h2_pool.tile([P, 512], f32r, tag="h2")
                    nc.vector.tensor_tensor(out=h2t, in0=ps_pv, in1=Rsb, op=OP.mult)
                    h2.append(h2t)
                # output projection + bias + residual
                for oi in range(CT):
                    ps_o = psum_mm.tile([P, 512], f32, tag="ps_mm")
                    for ci in range(CT):
                        nc.tensor.matmul(
                            ps_o,
                            wT["wp"][ci][:, oi * P : (oi + 1) * P],
                            h2[ci],
                            start=(ci == 0),
                            stop=(ci == CT - 1),
                        )
                    ot = out_pool.tile([P, 512], f32, tag="out")
                    nc.vector.scalar_tensor_tensor(
                        out=ot,
                        in0=ps_o,
                        scalar=bp_sb[:, oi : oi + 1],
                        in1=x_res[oi],
                        op0=OP.add,
                        op1=OP.add,
                    )
                    nc.gpsimd.dma_start(
                        out=out_d[oi * P : (oi + 1) * P, qb * 512 : (qb + 1) * 512],
                        in_=ot,
                    )
    nc.finalize()
    return nc


_NC = None


def _get_nc():
    global _NC
    if _NC is None:
        _NC = build()
    return _NC


def _consts():
    sel = np.zeros((P, GPT), np.float32)
    for rr in range(P):
        sel[rr, rr // GSZ] = 1.0
    selT = sel.T.copy()
    ident = np.eye(P, dtype=np.float32)
    return sel, selT, ident


def make_in_maps(inputs):
    x = np.ascontiguousarray(np.asarray(inputs["x"], dtype=np.float32))
    common = {}
    for nm in ("gn_scale", "gn_bias", "wq", "bq", "wk", "bk", "wv", "bv", "wp", "bp"):
        common[nm] = np.ascontiguousarray(np.asarray(inputs[nm], dtype=np.float32))
    sel, selT, ident = _consts()
    common["sel"] = sel
    common["selT"] = selT
    common["ident"] = ident
    return [dict(common, x=x[b]) for b in range(B)]


def kernel(**inputs):
    nc = _get_nc()
    in_maps = make_in_maps(inputs)
    res = run_bass_kernel_spmd(nc, in_maps, core_ids=list(range(B)))
    out = np.stack([res.results[b]["out"] for b in range(B)], axis=0)
    return out.astype(np.float32)
